# revision 90
# baseline (speedup 1.0000x reference)
"""MoE-routing (squeeze-excitation, K=4 conv1x1 experts) — Trainium2 Bass kernel, v5.

v5 over v4 (54798 -> 54616 ns): asymmetric kernel-tail fine units (768+256
cols instead of 512+512) — same sigmoid-stream length, but the post-last-
sigmoid serial drain (mul + DMA issue + transfer) runs on a 256-col unit;
b2e matmuls emitted inside routing_head right behind wv/s128 and
srecb/bias2 interleaved after fold-cj0 in DVE order (robustness: bias2 can
never queue behind pass-2 matmuls).

v4 over v3 (56114 -> 54798 ns): s broadcast to all 128 partitions via a
single ones[K,128] matmul so 1/s and bias2 are plain DVE ops; deferred
pass-1 relus data-gated on the routing chain (0-vector max() operand) so the
greedy scheduler cannot convoy them into the chain's semaphore holes; b1's
scans/routing prioritized so its chain clears the DVE queue before b0's
sigmoid stream ends (seamless 17.1->50.5us ACT stream); smaller final b0
wave (256 cols).  Floor: 1.97us DMA head + 47.43us serial DMA + 1.49us tail
= 50.9us; the residual ~3.9us is the sigmoid-production-paced store tail.
Explored and rejected: alternating dual-PSUM-pool 1536-wide sigmoids
(54867 — the ~940ns single-buffer pool turnaround always surfaces as an ACT
gap wherever adjacent units shrink below it), ACT/GPSIMD-issued tail stores
(54885/55327 — SP's issue pipe is shortest).  Original v3 notes below.
"""
"""

Strategy vs the fp32 baseline (111.7us):
  * bf16 end-to-end on the bulk path: x converted to bf16 on the host, res
    written bf16 (host converts back).  Halves HBM traffic (33.5MB -> ~17MB
    per core => ~47us DMA floor) and runs every PE matmul at 1 cyc/row.
  * Batched DMAs via 3D access patterns ([128, 4chunks, cols] per
    instruction): ~25 DMA instructions instead of 97.
  * pass-2 PSUM tiles are 2 banks wide so each sigmoid ACT instruction
    covers 1024 columns; sigmoid output is written bf16 to SBUF so the final
    x*attn multiply runs in the DVE 2x packed mode; routing scans run in the
    DVE 4x packed mode with per-chunk dump tiles (avoids WAW serialization).
  * single ACT table set: exp() in the softmax is computed via sigmoid
    (e^z = s/(1-s)), and the table is preloaded with a dummy sigmoid, so
    relu/sigmoid never force a mid-stream table reload.
  * softmax kept UNNORMALIZED through the folds: w2p = w2 * expand(e),
    bias2 = (e @ b2) / s, and the 1/s normalization rides the sigmoid's
    per-partition `scale` operand — shortens the routing critical chain.

Schedule: one seamless ACT sigmoid stream across both batches (ACT is the
pacing engine at ~33us); batch-1's load/scans/pass-1 are slotted between
batch-0's pass-2 groups, with its pass-2 matmuls given scheduler priority and
batch-0's trailing multiplies deferred so the batch-1 routing chain clears the
DVE queue in time.  The kernel tail is fine-grained (512-col sig/mul/out
units) to shorten the last-mul -> last-DMA drain.

Sharding: data-parallel over batch B=16 across 8 cores (2 per core), params
replicated.  Per core x is DMA'd once, kept SBUF-resident, multiplied
in-place, written once.  TimelineSim: 56114 ns (fp32 baseline: 111754 ns).
"""

import numpy as np
import ml_dtypes

import concourse.bass as bass
import concourse.bacc as bacc
import concourse.mybir as mybir
import concourse.tile as tile
from concourse.bass_utils import run_bass_kernel_spmd

N_CORES = 8
B, C, H, W = 16, 512, 64, 64
HW = H * W                  # 4096
K, D = 4, 32
KD = K * D                  # 128
P = 128
BPC = B // N_CORES          # 2 batches per core
NCH = C // P                # 4 channel chunks
TT = 512                    # matmul moving free-dim (one PSUM bank, fp32 out)
NT = HW // TT               # 8 t-tiles
TP = 2 * TT                 # pass-2 ACT read width (2 PSUM banks)
NTP = HW // TP              # 4

F32 = mybir.dt.float32
BF16 = mybir.dt.bfloat16
AF = mybir.ActivationFunctionType
ALU = mybir.AluOpType

# x DMA waves per batch (column ranges); b0's tail is split so the last
# scans (which gate routing) start sooner.  The final b0 wave is kept tiny
# (128 cols) so the post-arrival scan chain is short.
WAVES0 = [(0, 1024), (1024, 2176), (2176, 3072), (3072, 3584), (3584, 3840), (3840, 4096)]
WAVES1 = [(0, 1024), (1024, 2176), (2176, 3328), (3328, 4096)]

# ---- schedule tunables ----
RELU0_DVE = 0     # batch-0 relu tiles on DVE (rest ACT; ACT is idle in phase A)
RELU1_DVE = 8     # batch-1 relu tiles on DVE (ACT is saturated in phase B)
SCAN1_GP = 2      # per wave, how many of batch-1's 4 scans go to GPSIMD
OUT_GRAN = 1      # chunks per out-DMA (4 = one DMA per tp)
OUT_GRAN_LAST = 1
SG_BUFS = 16      # sigmoid-output ring depth (SBUF is cheap; deep ring lets muls trail)
MUL_GP = set()    # (b, tp) groups whose final muls run on GPSIMD instead of DVE
B1_AFTER = False   # emit b1's interleaved wave after pass2(b0, tp) (else before)


def build_bass():
    nc = bacc.Bacc("TRN2", target_bir_lowering=False)

    xs = nc.dram_tensor("xs", [BPC, C, HW], BF16, kind="ExternalInput")
    w1t = nc.dram_tensor("w1t", [P, NCH * KD + 16], BF16, kind="ExternalInput")
    w2t = nc.dram_tensor("w2t", [KD, C], BF16, kind="ExternalInput")
    wf32 = nc.dram_tensor("wf32", [P, 17], F32, kind="ExternalInput")
    wrt = nc.dram_tensor("wrt", [K, 778], F32, kind="ExternalInput")
    res = nc.dram_tensor("res", [BPC, C, HW], BF16, kind="ExternalOutput")

    with tile.TileContext(nc) as tc:
        with (
            tc.tile_pool(name="persist", bufs=1) as pp,
            tc.tile_pool(name="sg", bufs=SG_BUFS) as sgp,
        ):
            # PSUM pools allocated bottom-up: pop (4 banks, lives the whole
            # kernel), php (2 banks) and rtp (2 banks) released mid-kernel so
            # popC (4 banks, 2048-wide b1 pass-2 tiles) can take their space.
            pop = tc.alloc_tile_pool(name="po", bufs=2, space="PSUM")
            php = tc.alloc_tile_pool(name="ph", bufs=2, space="PSUM")
            rtp = tc.alloc_tile_pool(name="rt", bufs=2, space="PSUM")
            # ---- persistent SBUF tiles ----
            xt, hid, ysum = {}, {}, {}
            NW = {0: len(WAVES0), 1: len(WAVES1)}
            for b in range(BPC):
                xt[b] = pp.tile([P, NCH * HW], BF16, tag=f"x{b}", name=f"x{b}")
                hid[b] = pp.tile([KD, HW], BF16, tag=f"hid{b}", name=f"hid{b}")
                ysum[b] = pp.tile([P, NCH, NW[b]], F32, tag=f"ys{b}", name=f"ys{b}")
            xv = {b: xt[b].rearrange("p (j w) -> p j w", j=NCH) for b in range(BPC)}

            w1sb = pp.tile([P, NCH * KD + 16], BF16, tag="w1", name="w1sb")
            w1v = w1sb[:, 0:NCH * KD].rearrange("p (j m) -> p j m", j=NCH)
            fcw16 = w1sb[:, NCH * KD:NCH * KD + 16].rearrange(
                "p (j k) -> p j k", j=NCH
            )
            w2sb = pp.tile([KD, C], BF16, tag="w2", name="w2sb")
            fsb = pp.tile([P, 17], F32, tag="wf", name="fsb")
            fcwt = fsb[:, 0:16].rearrange("p (j k) -> p j k", j=NCH)
            b1v = fsb[:, 16:17]
            rsb = pp.tile([K, 778], F32, tag="wrt", name="rsb")
            fcb = rsb[:, 0:1]
            emat = rsb[:, 2:130]
            b2t = rsb[:, 134:646]
            ones4x128 = rsb[:, 646:774]       # all-ones [K, 128]
            eye4 = rsb[:, 774:778]

            rdump_v = pp.tile([P, NCH, 1280], BF16, tag="rdv", name="rdump_v")
            rdump_a = pp.tile([P, 256], BF16, tag="rda", name="rdump_a")
            rdump_g = pp.tile([P, 1024], BF16, tag="rdg", name="rdump_g")
            scr_act = pp.tile([1, 2], F32, tag="scr", name="scr_act")

            # ---- DMA helpers ----
            waves = {0: WAVES0, 1: WAVES1}

            def dma_x(b, q):
                c0, c1 = waves[b][q]
                nc.sync.dma_start(
                    out=xv[b][:, :, c0:c1],
                    in_=xs[b].rearrange("(j p) w -> p j w", p=P)[:, :, c0:c1],
                )

            # ---- pass 1: hid = relu(W1 @ x + b1) ----
            relu_dve = {0: RELU0_DVE, 1: RELU1_DVE}

            hp_defer = {}

            def pass1_mm(b, t):
                hp = php.tile([KD, TT], F32, tag="ph", name="hp")
                for j in range(NCH):
                    nc.tensor.matmul(
                        hp,
                        lhsT=w1v[:, j, :],
                        rhs=xv[b][:, j, t * TT:(t + 1) * TT],
                        start=(j == 0),
                        stop=(j == NCH - 1),
                    )
                hp_defer[b, t] = hp
                return hp

            def pass1_relu(b, t, gate=None):
                # gate: optional [P,1] zero tile used as the max() operand so
                # the relu acquires a data dependency on the routing chain —
                # keeps the greedy scheduler from slotting this long op into
                # DVE idle gaps ahead of the chain (DVE queue is in-order).
                hp = hp_defer.pop((b, t))
                dst = hid[b][:, t * TT:(t + 1) * TT]
                if gate is not None or t < relu_dve[b]:
                    nc.vector.tensor_scalar(
                        out=dst, in0=hp, scalar1=b1v,
                        scalar2=0.0 if gate is None else gate,
                        op0=ALU.add, op1=ALU.max,
                    )
                else:
                    nc.scalar.activation(out=dst, in_=hp, func=AF.Relu, bias=b1v)

            def pass1_t(b, t, relu_prio=None, gate=None):
                pass1_mm(b, t)
                if gate is not None:
                    pass1_relu(b, t, gate=gate)
                elif relu_prio is not None:
                    with tc.high_priority(offset=relu_prio):
                        pass1_relu(b, t)
                else:
                    pass1_relu(b, t)

            # ---- routing scans + partial fc matmuls, per wave ----
            r_ps = {}

            def scans(b, q, act_last=False):
                c0, c1 = waves[b][q]
                for j in range(NCH):
                    if act_last and j == NCH - 1:
                        # run the last chunk's scan on the (idle) ACT engine in
                        # parallel with the DVE scans — shaves the serial tail
                        # that gates batch-0's routing sigmoid
                        nc.scalar.activation(
                            out=rdump_a[:, 0:c1 - c0],
                            in_=xv[b][:, j, c0:c1],
                            func=AF.Copy,
                            accum_out=ysum[b][:, j, q:q + 1],
                        )
                        continue
                    nc.vector.tensor_scalar(
                        out=rdump_v[:, j, 0:c1 - c0],
                        in0=xv[b][:, j, c0:c1],
                        scalar1=1.0,
                        scalar2=0.0,
                        op0=ALU.mult,
                        op1=ALU.add,
                        accum_out=ysum[b][:, j, q:q + 1],
                    )

            def rt_mms(b, q):
                if q == 0:
                    r_ps[b] = rtp.tile([K, 1], F32, tag="rt", name="r_ps")
                for j in range(NCH):
                    nc.tensor.matmul(
                        r_ps[b],
                        lhsT=fcwt[:, j, :],
                        rhs=ysum[b][:, j, q:q + 1],
                        start=(q == 0 and j == 0),
                        stop=(q == NW[b] - 1 and j == NCH - 1),
                        skip_group_check=True,
                    )

            # ---- routing tail: unnormalized softmax folds ----
            # sg = sigmoid(r + fcb); e = sg/(1-sg) = exp(r + fcb)
            # w2p = w2 * expand(e)   (pass-2 lhsT; unnormalized)
            # bias2 = (e @ b2) / s;  sigmoid scale operand carries 1/s.
            # s is broadcast to all 128 partitions directly via a ones[K,128]
            # matmul so 1/s and bias2 are single DVE ops (no PE/ACT ping-pong
            # on the critical chain).
            w2p, bias2, srecb, e_sbs = {}, {}, {}, {}

            def routing_head(b, rin=None, split_fold=True):
                # serial chain: sigmoid -> e = sg/(1-sg) -> expand -> w2 fold
                if rin is None:
                    rin = r_ps[b]
                sgm = pp.tile([K, 1], F32, tag=f"sg{b}", name=f"sgm{b}")
                nc.scalar.activation(out=sgm, in_=rin, func=AF.Sigmoid, bias=fcb)
                onem = pp.tile([K, 1], F32, tag=f"om{b}", name=f"onem{b}")
                nc.vector.tensor_scalar(
                    out=onem, in0=sgm, scalar1=-1.0, scalar2=1.0,
                    op0=ALU.mult, op1=ALU.add,
                )
                onem_r = pp.tile([K, 1], F32, tag=f"omr{b}", name=f"onemr{b}")
                nc.vector.reciprocal(out=onem_r, in_=onem)
                e_sb = e_sbs[b] = pp.tile([K, 1], F32, tag=f"e{b}", name=f"e{b}")
                nc.vector.tensor_mul(e_sb, sgm, onem_r)
                wv_ps = rtp.tile([P, 1], F32, tag="rt", name="wv_ps")
                nc.tensor.matmul(wv_ps, lhsT=emat, rhs=e_sb, start=True, stop=True)
                s128_ps = rtp.tile([P, 1], F32, tag="rt", name="s128_ps")
                nc.tensor.matmul(
                    s128_ps, lhsT=ones4x128, rhs=e_sb, start=True, stop=True,
                    skip_group_check=True,
                )
                # b2e right behind wv/s128 on PE so bias2 never queues behind
                # pass-2 matmuls
                b2e_ps = rtp.tile([P, NCH], F32, tag="rt", name="b2e_ps")
                for cj in range(NCH):
                    nc.tensor.matmul(
                        b2e_ps[:, cj:cj + 1],
                        lhsT=b2t[:, cj * P:(cj + 1) * P],
                        rhs=e_sb,
                        start=True,
                        stop=True,
                        skip_group_check=True,
                    )
                # DVE order: fold cj0 first (gates the first pass-2 matmul),
                # then srecb/bias2 (gate the first sigmoid), then the rest
                w2p[b] = pp.tile([KD, C], BF16, tag=f"w2p{b}", name=f"w2p{b}")
                if split_fold:
                    nc.vector.tensor_scalar_mul(
                        w2p[b][:, 0:P], w2sb[:, 0:P], wv_ps
                    )
                else:
                    nc.vector.tensor_scalar_mul(w2p[b], w2sb, wv_ps)
                srecb[b] = pp.tile([P, 1], F32, tag=f"srb{b}", name=f"srecb{b}")
                nc.vector.reciprocal(out=srecb[b], in_=s128_ps)
                bias2[b] = pp.tile([P, NCH], F32, tag=f"b2{b}", name=f"bias2{b}")
                nc.vector.tensor_scalar_mul(bias2[b], b2e_ps, srecb[b])
                if split_fold:
                    for cj in range(1, NCH):
                        nc.vector.tensor_scalar_mul(
                            w2p[b][:, cj * P:(cj + 1) * P],
                            w2sb[:, cj * P:(cj + 1) * P],
                            wv_ps,
                        )

            def routing_rest(b):
                pass

            def routing_tail(b):
                routing_head(b)

            # ---- pass 2: res = x * sigmoid((w2p @ hid) / s + bias2) ----
            mul_defer = []

            def pass2_mul(b, tp, cj, sg, gate=None, gp=False):
                h0 = tp * TP
                xsl = xv[b][:, cj, h0:h0 + TP]
                if gp:
                    # deferred muls run on the idle GPSIMD engine: slower, but
                    # off the congested DVE, and their stores drip into the
                    # DMA gaps of the b1 production-paced store phase
                    nc.gpsimd.tensor_mul(xsl, sg, xsl)
                elif gate is None:
                    nc.vector.tensor_mul(xsl, sg, xsl)
                else:
                    # same product, but (sg + 0-gate) picks up a data dep on
                    # the routing chain so the scheduler cannot convoy this
                    # long op into the chain's semaphore holes
                    nc.vector.scalar_tensor_tensor(
                        out=xsl, in0=sg, scalar=gate, in1=xsl,
                        op0=ALU.add, op1=ALU.mult,
                    )

            def pass2_tile(b, tp, cj, defer_mul=False):
                h0 = tp * TP
                po = pop.tile([P, TP], F32, tag="po", name="po")
                with tc.high_priority(offset=500):
                    for h in range(2):
                        nc.tensor.matmul(
                            po[:, h * TT:(h + 1) * TT],
                            lhsT=w2p[b][:, cj * P:(cj + 1) * P],
                            rhs=hid[b][:, h0 + h * TT:h0 + (h + 1) * TT],
                            start=True,
                            stop=True,
                        )
                # deferred (GPSIMD-muled) chunks get their own sg ring so the
                # slow Pool readers never back up the main sigmoid stream
                tag = "sgGP" if defer_mul else "sg"
                sg = sgp.tile([P, TP], BF16, tag=tag, name=tag)
                nc.scalar.activation(
                    out=sg, in_=po, func=AF.Sigmoid,
                    bias=bias2[b][:, cj:cj + 1], scale=srecb[b],
                )
                if defer_mul:
                    mul_defer.append((b, tp, cj, sg))
                else:
                    pass2_mul(b, tp, cj, sg)

            def flush_muls(gate=None, gp_tps=()):
                while mul_defer:
                    b, tp, cj, sg = mul_defer.pop(0)
                    if tp in gp_tps:
                        pass2_mul(b, tp, cj, sg, gp=True)
                    else:
                        pass2_mul(b, tp, cj, sg, gate=gate)

            def out_dma(b, tp, gran):
                h0 = tp * TP
                rv = res[b].rearrange("(j p) w -> p j w", p=P)
                for j0 in range(0, NCH, gran):
                    nc.sync.dma_start(
                        out=rv[:, j0:j0 + gran, h0:h0 + TP],
                        in_=xv[b][:, j0:j0 + gran, h0:h0 + TP],
                    )

            def out_dma1(b, tp, cj, halves=False):
                h0 = tp * TP
                rv = res[b].rearrange("(j p) w -> p j w", p=P)
                if halves:
                    for h in range(2):
                        nc.sync.dma_start(
                            out=rv[:, cj:cj + 1, h0 + h * TT:h0 + (h + 1) * TT],
                            in_=xv[b][:, cj:cj + 1, h0 + h * TT:h0 + (h + 1) * TT],
                        )
                else:
                    nc.sync.dma_start(
                        out=rv[:, cj:cj + 1, h0:h0 + TP],
                        in_=xv[b][:, cj:cj + 1, h0:h0 + TP],
                    )

            # ================= schedule =================
            # phase A: load + pass1(b0) + routing(b0)
            dma_x(0, 0)
            nc.sync.dma_start(out=w1sb, in_=w1t[:, :])
            nc.sync.dma_start(out=fsb, in_=wf32[:, :])
            nc.sync.dma_start(out=rsb, in_=wrt[:, :])
            # Preload the single ACT table set ("sigmoid_and_others" covers
            # relu/sigmoid) while the queue is empty.
            nc.vector.memset(scr_act[0:1, 0:1], 0.0)
            nc.scalar.activation(
                out=scr_act[0:1, 1:2], in_=scr_act[0:1, 0:1], func=AF.Sigmoid
            )
            # wave -> fully-covered pass1 t-tiles (relu of the last two tiles
            # is deferred until after routing_head so it can't delay the
            # routing sigmoid on the in-order ACT queue)
            done_cols = 0
            emitted_t = 0
            for q in range(NW[0]):
                if q > 0:
                    dma_x(0, q)
                with tc.high_priority():
                    scans(0, q)
                    rt_mms(0, q)
                done_cols = waves[0][q][1]
                while (emitted_t + 1) * TT <= done_cols:
                    t = emitted_t
                    pass1_mm(0, t)
                    if t < NT - 2:
                        pass1_relu(0, t)
                    emitted_t += 1
            nc.sync.dma_start(out=w2sb, in_=w2t[:, :])
            # offset far past pass2's offset=500 so the routing chain's PE
            # matmuls (wv_ps/s_ps/b2e/sr_ps) beat the first po matmuls in the
            # PE queue — sr_ps gates bias2/srecb which gate the first sigmoid.
            with tc.high_priority(offset=100000):
                routing_head(0)
                routing_rest(0)
                # zero vector derived from the chain end; gates deferred relus
                zro = pp.tile([P, 1], F32, tag="zro", name="zro")
                nc.vector.tensor_scalar(
                    out=zro, in0=srecb[0], scalar1=0.0, scalar2=0.0,
                    op0=ALU.mult, op1=ALU.add,
                )
            # deferred relus on DVE (ACT must stay clear for the sigmoid
            # stream; DVE is idle right after the routing chain)
            pass1_relu(0, NT - 2, gate=zro)
            pass1_relu(0, NT - 1, gate=zro)
            for q in range(NW[1]):
                dma_x(1, q)

            # phases B/C: one continuous pass-2 stream over groups
            #   (b0,tp0..tp2) (b1,tp0) (b0,tp3) (b1,tp1..tp3)
            # The (b1,tp0)/(b0,tp3) swap keeps the ACT sigmoid stream and the
            # PSUM po-slot rotation gap-free across the batch transition.
            # b1's pass-1/scan/routing work is slotted between groups.
            def pass2_fine_unit(b, cj, c0, c1):
                # fine sig -> mul -> out chain for the kernel tail; the very
                # last unit is 256 cols so the post-sigmoid serial drain
                # (mul + DMA issue + transfer) is as short as possible
                po = pop.tile([P, TP], F32, tag="po", name="po")
                for h0 in range(c0, c1, TT):
                    h1 = min(h0 + TT, c1)
                    nc.tensor.matmul(
                        po[:, h0 - c0:h1 - c0],
                        lhsT=w2p[b][:, cj * P:(cj + 1) * P],
                        rhs=hid[b][:, h0:h1],
                        start=True,
                        stop=True,
                    )
                sg = sgp.tile([P, TP], BF16, tag="sg", name="sg")
                nc.scalar.activation(
                    out=sg[:, 0:c1 - c0], in_=po[:, 0:c1 - c0], func=AF.Sigmoid,
                    bias=bias2[b][:, cj:cj + 1], scale=srecb[b],
                )
                xsl = xv[b][:, cj, c0:c1]
                nc.vector.tensor_mul(xsl, sg[:, 0:c1 - c0], xsl)
                rv = res[b].rearrange("(j p) w -> p j w", p=P)
                nc.sync.dma_start(
                    out=rv[:, cj:cj + 1, c0:c1],
                    in_=xv[b][:, cj:cj + 1, c0:c1],
                )

            def pass2_group(b, tp, fine=False, defer_mul=False):
                for cj in range(NCH):
                    if fine and cj == NCH - 1:
                        pass2_fine_unit(b, cj, tp * TP, tp * TP + 768)
                        pass2_fine_unit(b, cj, tp * TP + 768, (tp + 1) * TP)
                        continue
                    pass2_tile(b, tp, cj, defer_mul=defer_mul)
                    if fine:
                        out_dma1(b, tp, cj)
                if not fine and not defer_mul:
                    out_dma(b, tp, OUT_GRAN)

            scans(1, 0)
            pass2_group(0, 0)
            scans(1, 1)
            rt_mms(1, 0)
            pass1_t(1, 0)
            pass1_t(1, 1)
            pass2_group(0, 1)
            with tc.high_priority(offset=100000):
                scans(1, 2)
                rt_mms(1, 1)
                rt_mms(1, 2)
            pass2_group(0, 2, defer_mul=True)
            with tc.high_priority(offset=100000):
                for q in range(3, NW[1]):
                    scans(1, q)
                    rt_mms(1, q)
                routing_head(1, split_fold=False)
                routing_rest(1)
            pass2_group(0, 3, defer_mul=True)
            flush_muls()
            out_dma(0, 2, OUT_GRAN)
            out_dma(0, 3, OUT_GRAN)
            pass1_t(1, 2)
            pass1_t(1, 3)
            pass2_group(1, 0)
            pass1_t(1, 4)
            pass1_t(1, 5)
            pass2_group(1, 1)
            pass1_t(1, 6)
            pass1_t(1, 7)
            pass2_group(1, 2)
            pass2_group(1, 3, fine=True)

            rtp.release()
            php.release()
            pop.release()

    nc.compile()
    return nc


_NC_CACHE = None


def _get_nc():
    global _NC_CACHE
    if _NC_CACHE is None:
        _NC_CACHE = build_bass()
    return _NC_CACHE


def _prep_inputs(x, fc_w, fc_b, w1, b1, w2, b2):
    """Host-side dtype conversion, weight re-layout, per-core shards."""
    f = np.float32
    bf = ml_dtypes.bfloat16
    x = np.ascontiguousarray(x, dtype=f).reshape(B, C, HW).astype(bf)

    w1t = w1.transpose(2, 0, 1).reshape(C, KD)          # [c, kd]
    fcwt_h = (fc_w.T / HW).astype(f)                    # [c, k]
    fcw16_h = fcwt_h.reshape(NCH, P, K).transpose(1, 0, 2).reshape(P, 16)
    w1t16 = np.concatenate(
        [
            w1t.reshape(NCH, P, KD).transpose(1, 0, 2).reshape(P, NCH * KD),
            fcw16_h,
        ],
        axis=1,
    ).astype(bf)
    w1t16 = np.ascontiguousarray(w1t16)
    w2t16 = np.ascontiguousarray(w2.transpose(0, 2, 1).reshape(KD, C)).astype(bf)

    wf32 = np.zeros((P, 17), dtype=f)
    fcwt = (fc_w.T / HW).astype(f)                       # [c, k]
    wf32[:, 0:16] = fcwt.reshape(NCH, P, K).transpose(1, 0, 2).reshape(P, 16)
    wf32[:, 16] = b1.reshape(KD)

    wrt = np.zeros((K, 778), dtype=f)
    wrt[:, 774:778] = np.eye(K)
    wrt[:, 0] = fc_b
    wrt[:, 1] = 1.0
    wrt[:, 2:130] = np.kron(np.eye(K), np.ones((1, D)))
    wrt[:, 134:646] = b2
    wrt[:, 646:774] = 1.0

    shared = {"w1t": w1t16, "w2t": w2t16, "wf32": wf32, "wrt": wrt}
    in_maps = []
    for i in range(N_CORES):
        m = dict(shared)
        m["xs"] = np.ascontiguousarray(x[i * BPC:(i + 1) * BPC])
        in_maps.append(m)
    return in_maps


def run_on_device(inputs, trace=False):
    nc = _get_nc()
    in_maps = _prep_inputs(**inputs)
    r = run_bass_kernel_spmd(
        nc, in_maps, core_ids=list(range(N_CORES)), trace=trace
    )
    out = np.concatenate(
        [np.asarray(d["res"]).astype(np.float32) for d in r.results], axis=0
    )
    return out.reshape(B, C, H, W), r


def kernel(**inputs) -> np.ndarray:
    out, _ = run_on_device(inputs, trace=False)
    return out

